# revision 1
# baseline (speedup 1.0000x reference)
"""C2Q attention kernel for 8 TRN2 NeuronCores.

Math (per batch):
    u      = (o_q @ W.T + b) / sqrt(H)          [Tq, H]
    score  = o_c @ u.T                           [Tc, Tq]
    prob   = softmax_j(score masked at j>=q_len) [Tc, Tq]
    out    = (prob * (i < c_len)) @ o_q          [Tc, H]

Device layout choices (everything lands K-on-partitions with zero on-chip
transposes of activations):
    u computed as [o, j]  (lhsT = W.T[h, o] tile, rhs = o_qT[h, j])
    score computed TRANSPOSED e=[j, i] (lhsT = u[o, j-block], rhs = o_cT[o, i])
    exp via ACT with per-partition bias qb[j] in {0, -1e7}: masked -> exactly 0
    denominator d[1, i] = ones[j,1].T @ e  (matmul partition-reduce)
    1/d transposed to columns via K=1 matmuls, folded into context eviction
    context [i, h] = e[j, i-block].T @ o_q[j, h]   (natural output layout)
c_len row masking is applied host-side (those rows are zeroed, never read).
"""

import os
import sys

import numpy as np

if "/opt/trn_rl_repo" not in sys.path:
    sys.path.insert(0, "/opt/trn_rl_repo")

B, Tc, Tq, H = 32, 512, 512, 1024
N_CORES = 8
B_LOCAL = B // N_CORES
KT = H // 128  # contraction tiles over h (8)
OT = H // 128  # linear-output tiles over o (8)
JT = Tq // 128  # question-token tiles (4)
IT = Tc // 128  # context-token tiles (4)
HB = H // 512  # free-dim blocks for context matmul (2)
SCALE = 1.0 / 32.0  # 1/sqrt(H)
NEG = -1.0e7


def _build_program(b_local: int, use_f32r: bool = True):
    import concourse.bacc as bacc
    import concourse.mybir as mybir
    import concourse.tile as tile

    f32 = mybir.dt.float32
    # reduced-precision single-pass fp32 matmul format; every tensor feeding
    # an fp32r matmul must itself be typed fp32r end-to-end (BIR verifier)
    mdt = mybir.dt.float32r if use_f32r else mybir.dt.float32

    nc = bacc.Bacc("TRN2", debug=False)

    # Small per-partition constants ride as extra columns of the big slabs
    # (a standalone [128, few] DMA costs 128 descriptors for ~2KB and clogs
    # the DGE descriptor stream during the ramp):
    #   wt slab 0 cols 1024:1032 = bias/32 arranged [p, o_tile], col 1032 = 1.0
    #   oqT slab 7 cols 512:516  = exp-bias qb (0 / -1e7) arranged [p, j_tile]
    WTW = H + 16  # wt slab width (pad)
    QTW = Tq + 8  # oqT slab width (pad)
    CTW = Tc + 8  # ocT slab width (pad); slab 0 col Tc = ones
    f16 = mybir.dt.float16
    oqT_d = nc.declare_dram_parameter("oqT", [b_local, KT, 128, QTW], f16, isOutput=False)
    ocT_d = nc.declare_dram_parameter("ocT", [b_local, KT, 128, CTW], f16, isOutput=False)
    oqN_d = nc.declare_dram_parameter("oqN", [b_local, Tq, H], f16, isOutput=False)
    wt_d = nc.declare_dram_parameter("wt", [KT, 128, WTW], f16, isOutput=False)
    bias_d = nc.declare_dram_parameter("biasP", [128, OT], f32, isOutput=False)
    out_d = nc.declare_dram_parameter("out", [b_local, Tc, H], f32, isOutput=True)

    with tile.TileContext(nc) as tc:
        with (
            tc.tile_pool(name="const", bufs=1) as cpool,
            tc.tile_pool(name="inp", bufs=2) as ipool,
            tc.tile_pool(name="work", bufs=1) as wpool,
            tc.tile_pool(name="outp", bufs=3) as opool,
            tc.tile_pool(name="ps_u", bufs=2, space="PSUM") as ps_u,
            tc.tile_pool(name="ps_s", bufs=2, space="PSUM") as ps_s,
            tc.tile_pool(name="ps_c", bufs=3, space="PSUM") as ps_c,
            tc.tile_pool(name="ps_d", bufs=1, space="PSUM") as ps_d,
        ):
            ones_s = cpool.tile([1, 1], f32)
            nc.vector.memset(ones_s, 1.0)

            # W tiles: one tile per k so the first matmuls depend only on the
            # first slices; DMAs interleaved with batch-0 oqT below.
            wt_k = [cpool.tile([128, WTW], f16, tag=f"wt{k}", name=f"wt{k}") for k in range(KT)]
            biasP = cpool.tile([128, OT], f32)

            for b in range(b_local):
                # per-k tiles keep DMA->matmul deps fine-grained during ramp
                oqT_k = [ipool.tile([128, QTW], f16, tag=f"oqT{k}", name=f"oqT{k}_{b}") for k in range(KT)]
                ocT_k = [ipool.tile([128, CTW], f16, tag=f"ocT{k}", name=f"ocT{k}_{b}") for k in range(KT)]
                oqN = ipool.tile([128, JT, H], f16, tag="oqN")
                qb = oqT_k[KT - 1][:, Tq : Tq + JT]
                ones = ocT_k[0][:, Tc : Tc + 1]
                if b == 0:
                    # one tiny DMA (~0.65us of descriptor stream) ahead of the
                    # bulk: the first Linear evictions depend on it
                    nc.sync.dma_start(out=biasP, in_=bias_d[:, :])
                for k in range(KT):
                    if b == 0:
                        nc.sync.dma_start(out=wt_k[k], in_=wt_d[k])
                    nc.sync.dma_start(out=oqT_k[k], in_=oqT_d[b, k])
                for k in range(KT):
                    nc.sync.dma_start(out=ocT_k[k], in_=ocT_d[b, k])
                for j in range(JT):
                    nc.sync.dma_start(
                        out=oqN[:, j, :], in_=oqN_d[b, j * 128 : (j + 1) * 128, :]
                    )

                # ---- Linear: u[o, j] = W'@o_q.T + b'  (W', b' pre-scaled by
                # 1/32 on host). For batch 0 the contraction is split into
                # quarters so the PE has runnable matmuls as soon as each
                # ~1.5MB of wt/oqT has streamed in.
                u = wpool.tile([128, OT, Tq], f16, tag="u")
                for o in range(OT):
                    ups = ps_u.tile([128, Tq], f32, tag="ups")
                    for k in range(KT):
                        nc.tensor.matmul(
                            ups,
                            wt_k[k][:, o * 128 : (o + 1) * 128],
                            oqT_k[k][:, :Tq],
                            start=(k == 0),
                            stop=(k == KT - 1),
                        )
                    nc.vector.tensor_scalar(
                        out=u[:, o, :],
                        in0=ups,
                        scalar1=biasP[:, o : o + 1],
                        scalar2=None,
                        op0=mybir.AluOpType.add,
                    )

                # ---- score_T + exp: e[j, i] = exp((u.T @ o_cT)/32 + qbias[j]),
                # with the denominator accumulation d[1, i] = sum_j e[j, i]
                # interleaved one step behind so its chain latency hides ----
                dps = ps_d.tile([1, Tc], f32, tag="dmisc", name=f"dps_{b}")
                e_tiles = []
                for jt in range(JT):
                    sps = ps_s.tile([128, Tc], f32, tag="sps")
                    for o in range(OT):
                        nc.tensor.matmul(
                            sps,
                            u[:, o, jt * 128 : (jt + 1) * 128],
                            ocT_k[o][:, :Tc],
                            start=(o == 0),
                            stop=(o == OT - 1),
                        )
                    e = wpool.tile([128, Tc], f16, tag=f"e{jt}")
                    nc.scalar.activation(
                        out=e,
                        in_=sps,
                        func=mybir.ActivationFunctionType.Exp,
                        bias=qb[:, jt : jt + 1],
                        scale=SCALE,
                    )
                    e_tiles.append(e)
                    if jt >= 1:
                        nc.tensor.matmul(
                            dps,
                            ones,
                            e_tiles[jt - 1],
                            start=(jt == 1),
                            stop=False,
                            skip_group_check=True,
                        )
                nc.tensor.matmul(
                    dps,
                    ones,
                    e_tiles[JT - 1],
                    start=False,
                    stop=True,
                    skip_group_check=True,
                )

                osb_tiles = {}

                def ctx_group(it, hb):
                    if it not in osb_tiles:
                        osb_tiles[it] = opool.tile(
                            [128, H], f32, tag="osb", name=f"osb{it}_{b}"
                        )
                    cps = ps_c.tile([128, 512], f32, tag="cps", name=f"cps{it}{hb}_{b}")
                    for jt in range(JT):
                        nc.tensor.matmul(
                            cps,
                            e_tiles[jt][:, it * 128 : (it + 1) * 128],
                            oqN[:, jt, hb * 512 : (hb + 1) * 512],
                            start=(jt == 0),
                            stop=(jt == JT - 1),
                        )
                    return cps

                def ctx_evict(it, hb, cps, r):
                    osb = osb_tiles[it]
                    nc.vector.tensor_scalar(
                        out=osb[:, hb * 512 : (hb + 1) * 512],
                        in0=cps,
                        scalar1=r,
                        scalar2=None,
                        op0=mybir.AluOpType.mult,
                    )
                    nc.sync.dma_start(
                        out=out_d[
                            b, it * 128 : (it + 1) * 128, hb * 512 : (hb + 1) * 512
                        ],
                        in_=osb[:, hb * 512 : (hb + 1) * 512],
                    )

                # first ctx group runs while the d copy drains on DVE
                cps00 = ctx_group(0, 0)
                dsb = wpool.tile([1, Tc], f32, tag="dsb")
                nc.vector.tensor_copy(out=dsb, in_=dps)

                # transpose 1/d to per-partition columns via K=1 matmuls
                r_cols = []
                for it in range(IT):
                    dcps = ps_d.tile([128, 1], f32, tag="dmisc", name=f"dcps{it}_{b}")
                    nc.tensor.matmul(
                        dcps,
                        dsb[:, it * 128 : (it + 1) * 128],
                        ones_s[0:1, 0:1],
                        start=True,
                        stop=True,
                    )
                    r = wpool.tile([128, 1], f32, tag=f"r{it}")
                    nc.vector.reciprocal(out=r, in_=dcps)
                    r_cols.append(r)

                cps01 = ctx_group(0, 1)
                ctx_evict(0, 0, cps00, r_cols[0])
                ctx_evict(0, 1, cps01, r_cols[0])
                for it in range(1, IT):
                    for hb in range(HB):
                        cps = ctx_group(it, hb)
                        ctx_evict(it, hb, cps, r_cols[it])

    nc.compile()
    return nc


def _host_inputs(o_c, o_q, W, b, q_lengths):
    """Build the per-core input maps (host-side sharding + re-layout).

    Linear operands (W, o_qT) ship as fp16 (same PE rate, half the
    ramp-critical DMA bytes); the 1/sqrt(H) scale is applied later as the
    Exp activation's scale argument, so W keeps its natural fp16 range.
    """
    WTW, QTW, CTW = H + 16, Tq + 8, Tc + 8
    NEG16 = np.float16(-60000.0)  # exp(x - 60000) == 0 exactly in fp32
    wt_host = np.zeros((KT, 128, WTW), np.float16)
    wt_host[:, :, :H] = W.T.reshape(KT, 128, H)
    bias_host = np.ascontiguousarray(b.reshape(OT, 128).T)  # [128, o_tile] f32
    jidx = np.arange(JT)[None, :] * 128 + np.arange(128)[:, None]  # [128, JT]
    in_maps = []
    for c in range(N_CORES):
        sl = slice(c * B_LOCAL, (c + 1) * B_LOCAL)
        oq = np.ascontiguousarray(o_q[sl].astype(np.float16))
        ocT = np.zeros((B_LOCAL, KT, 128, CTW), np.float16)
        ocT[:, :, :, :Tc] = o_c[sl].transpose(0, 2, 1).reshape(B_LOCAL, KT, 128, Tc)
        ocT[:, 0, :, Tc] = 1.0  # ones column for the denominator matmul
        oqT = np.zeros((B_LOCAL, KT, 128, QTW), np.float16)
        oqT[:, :, :, :Tq] = (
            o_q[sl].transpose(0, 2, 1).reshape(B_LOCAL, KT, 128, Tq)
        )
        for lb in range(B_LOCAL):
            ql = int(q_lengths[c * B_LOCAL + lb])
            # qb (exp bias: 0 valid / -60000 masked) rides in the last slab
            oqT[lb, KT - 1, :, Tq : Tq + JT] = np.where(
                jidx < ql, np.float16(0.0), NEG16
            )
        in_maps.append(
            {"oqT": oqT, "ocT": ocT, "oqN": oq, "wt": wt_host, "biasP": bias_host}
        )
    return in_maps


def kernel(**inputs) -> np.ndarray:
    o_c = np.asarray(inputs["o_c"], dtype=np.float32)
    o_q = np.asarray(inputs["o_q"], dtype=np.float32)
    W = np.asarray(inputs["W"], dtype=np.float32)
    b = np.asarray(inputs["b"], dtype=np.float32)
    q_lengths = np.asarray(inputs["q_lengths"]).astype(np.int64)
    c_lengths = np.asarray(inputs["c_lengths"]).astype(np.int64)

    from concourse.bass_utils import run_bass_kernel_spmd

    in_maps = _host_inputs(o_c, o_q, W, b, q_lengths)
    nc = _build_program(B_LOCAL)

    trace = bool(int(os.environ.get("KERNEL_TRACE", "0")))
    res = run_bass_kernel_spmd(
        nc, in_maps, core_ids=list(range(N_CORES)), trace=trace
    )
    if trace:
        kernel.last_results = res

    out = np.zeros((B, Tc, H), dtype=np.float32)
    for c in range(N_CORES):
        dev = res.results[c]["out"]
        for lb in range(B_LOCAL):
            g = c * B_LOCAL + lb
            cl = int(c_lengths[g])
            out[g, :cl] = dev[lb, :cl]
    return out



# revision 3
# speedup vs baseline: 1.2182x; 1.2182x over previous
"""C2Q attention kernel for 8 TRN2 NeuronCores, ragged-aware.

Math (per batch):
    u      = o_q @ W.T + b                       [Tq, H]
    score  = (o_c @ u.T) / sqrt(H)               [Tc, Tq]
    prob   = softmax_j(score masked at j>=q_len) [Tc, Tq]
    out    = (prob * (i < c_len)) @ o_q          [Tc, H]

Ragged scheme: lengths are in [Tq/2, Tq] x [Tc/2, Tc]. All 8 cores run one
SPMD program with 4 batch slots; slot s is compiled with budgets
Bq_s = max(q_len) / Bc_s = max(c_len) over the 8 batches assigned to that
slot (one per core). Batches are assigned to slots by a cost sort + local
search so budgets hug the actual lengths. Every matmul free dim and tile
count is sized to the slot budget:
    Linear  : 64 matmuls, N = Bq
    score   : 8 * jt_n matmuls, N = Bc, stationary M partial on last j tile
    context : 2 * it_n * jt_n matmuls, K partial on last j tile
    denom   : jt_n matmuls N = Bc; it_n K=1 transposes for 1/d
Device layout (everything K-on-partitions, no on-chip transposes):
    u computed as [o, j]  (lhsT = W.T[h, o] tile, rhs = o_qT[h, j])
    score computed TRANSPOSED e=[j, i] (lhsT = u[o, j-block], rhs = o_cT[o, i])
    exp via ACT with per-partition bias qb[j] in {0, -60000}: masked -> 0
    denominator d[1, i] = ones[j,1].T @ e  (matmul partition-reduce)
    1/d transposed to columns via K=1 matmuls, folded into context eviction
    context [i, h] = e[j, i-block].T @ o_q[j, h]
c_len row masking is applied host-side (rows >= c_len are never copied out).
"""

import os
import sys

import numpy as np

if "/opt/trn_rl_repo" not in sys.path:
    sys.path.insert(0, "/opt/trn_rl_repo")

B, Tc, Tq, H = 32, 512, 512, 1024
N_CORES = 8
N_SLOTS = B // N_CORES  # 4
KT = H // 128  # contraction tiles over h (8)
OT = H // 128  # linear-output tiles over o (8)
JT = Tq // 128  # max question-token tiles (4)
HB = H // 512  # free-dim blocks for context matmul (2)
SCALE = 1.0 / 32.0  # 1/sqrt(H)
WTW = H + 16  # wt slab width (pad)
QTW = Tq + 8  # oqT slab width (pad); qb rides at cols [Bq, Bq+jt_n)
CTW = Tc + 8  # ocT slab width (pad); ones column at col Bc of k=0 slab


def _ceil_div(a, b):
    return -(-a // b)


def _slot_cost(bq, bc):
    """Per-batch PE-time model (in 2.4GHz cycles) for budget (bq, bc)."""
    jt = max(1, _ceil_div(bq, 128))
    it = max(1, _ceil_div(bc, 128))
    lin = 64 * (bq + 112)
    score = 8 * jt * (bc + 112)
    ctx = 2 * it * jt * (512 + 112)
    den = jt * (bc + 16) + it * 382
    return lin + score + ctx + den


def _plan(q_lengths, c_lengths):
    """Assign batches to (core, slot) minimizing total slot-budget cost.

    Returns perm[slot][core] -> global batch idx, budgets[slot] = (Bq, Bc).
    """
    ql = np.clip(np.asarray(q_lengths, dtype=np.int64), 1, Tq)
    cl = np.clip(np.asarray(c_lengths, dtype=np.int64), 1, Tc)
    cost = np.array([_slot_cost(q, c) for q, c in zip(ql, cl)])
    order = np.argsort(cost, kind="stable")
    slots = [list(order[s * N_CORES : (s + 1) * N_CORES]) for s in range(N_SLOTS)]

    def total(sl):
        t = 0
        for members in sl:
            bq = max(int(ql[b]) for b in members)
            bc = max(int(cl[b]) for b in members)
            t += _slot_cost(bq, bc)
        return t

    best = total(slots)
    improved = True
    while improved:
        improved = False
        for s1 in range(N_SLOTS):
            for s2 in range(s1 + 1, N_SLOTS):
                for i in range(N_CORES):
                    for j in range(N_CORES):
                        a, b_ = slots[s1][i], slots[s2][j]
                        slots[s1][i], slots[s2][j] = b_, a
                        t = total(slots)
                        if t < best:
                            best = t
                            improved = True
                        else:
                            slots[s1][i], slots[s2][j] = a, b_
    budgets = []
    for members in slots:
        bq = max(int(ql[b]) for b in members)
        bc = max(int(cl[b]) for b in members)
        budgets.append((bq, bc))
    # order slots so the cheapest runs first (fast ramp) -- keep budgets
    # aligned with membership
    idx = sorted(range(N_SLOTS), key=lambda s: _slot_cost(*budgets[s]))
    slots = [slots[s] for s in idx]
    budgets = [budgets[s] for s in idx]
    return slots, budgets


def _build_program(budgets):
    import concourse.bacc as bacc
    import concourse.mybir as mybir
    import concourse.tile as tile

    f32 = mybir.dt.float32
    f16 = mybir.dt.float16

    nc = bacc.Bacc("TRN2", debug=False)

    oqT_d = nc.declare_dram_parameter("oqT", [N_SLOTS, 128, KT, QTW], f16, isOutput=False)
    ocT_d = nc.declare_dram_parameter("ocT", [N_SLOTS, 128, KT, CTW], f16, isOutput=False)
    oqN_d = nc.declare_dram_parameter("oqN", [N_SLOTS, 128, JT, H], f16, isOutput=False)
    wt_d = nc.declare_dram_parameter("wt", [KT, 128, WTW], f16, isOutput=False)
    bias_d = nc.declare_dram_parameter("biasP", [128, OT], f32, isOutput=False)
    out_d = nc.declare_dram_parameter("out", [N_SLOTS, Tc, H], f32, isOutput=True)

    with tile.TileContext(nc) as tc:
        with (
            tc.tile_pool(name="const", bufs=1) as cpool,
            tc.tile_pool(name="inp", bufs=1) as ipool,
            tc.tile_pool(name="work", bufs=1) as wpool,
            tc.tile_pool(name="outp", bufs=3) as opool,
            tc.tile_pool(name="ps_u", bufs=2, space="PSUM") as ps_u,
            tc.tile_pool(name="ps_s", bufs=2, space="PSUM") as ps_s,
            tc.tile_pool(name="ps_c", bufs=3, space="PSUM") as ps_c,
            tc.tile_pool(name="ps_d", bufs=1, space="PSUM") as ps_d,
        ):
            ones_s = cpool.tile([1, 1], f32)
            nc.vector.memset(ones_s, 1.0)

            wt_k = [
                cpool.tile([128, WTW], f16, tag=f"wt{k}", name=f"wt{k}")
                for k in range(KT)
            ]
            biasP = cpool.tile([128, OT], f32)

            # --- per-slot geometry + input tiles, all DMAs issued upfront ---
            geo = []
            for s, (Bq, Bc) in enumerate(budgets):
                jt_n = max(1, _ceil_div(Bq, 128))
                it_n = max(1, _ceil_div(Bc, 128))
                QW = Bq + 8
                CW = Bc + 8
                if s == 0:
                    # per-k tiles keep DMA->matmul deps fine-grained during ramp
                    oqT = [
                        ipool.tile([128, QW], f16, tag=f"oqT0_{k}", name=f"oqT0_{k}")
                        for k in range(KT)
                    ]
                    ocT = [
                        ipool.tile([128, CW], f16, tag=f"ocT0_{k}", name=f"ocT0_{k}")
                        for k in range(KT)
                    ]
                else:
                    oqT = ipool.tile([128, KT, QW], f16, tag=f"oqT{s}")
                    ocT = ipool.tile([128, KT, CW], f16, tag=f"ocT{s}")
                oqN = ipool.tile([128, jt_n, H], f16, tag=f"oqN{s}")
                geo.append((Bq, Bc, jt_n, it_n, oqT, ocT, oqN))

            # DMA order: bias, interleaved wt/oqT(slot0), ocT(slot0), oqN(0),
            # then merged slabs for the remaining slots.
            nc.sync.dma_start(out=biasP, in_=bias_d[:, :])
            Bq0, Bc0, jt0, _, oqT0, ocT0, oqN0 = geo[0]
            for k in range(KT):
                nc.sync.dma_start(out=wt_k[k], in_=wt_d[k])
                nc.sync.dma_start(out=oqT0[k], in_=oqT_d[0, :, k, : Bq0 + 8])
            for k in range(KT):
                nc.sync.dma_start(out=ocT0[k], in_=ocT_d[0, :, k, : Bc0 + 8])
            nc.sync.dma_start(out=oqN0, in_=oqN_d[0, :, :jt0, :])
            for s in range(1, N_SLOTS):
                Bq, Bc, jt_n, it_n, oqT, ocT, oqN = geo[s]
                nc.sync.dma_start(out=oqT, in_=oqT_d[s, :, :, : Bq + 8])
                nc.sync.dma_start(out=ocT, in_=ocT_d[s, :, :, : Bc + 8])
                nc.sync.dma_start(out=oqN, in_=oqN_d[s, :, :jt_n, :])

            for s in range(N_SLOTS):
                Bq, Bc, jt_n, it_n, oqT, ocT, oqN = geo[s]

                def oqT_sl(k):
                    return oqT[k] if s == 0 else oqT[:, k, :]

                def ocT_sl(k):
                    return ocT[k] if s == 0 else ocT[:, k, :]

                qb = oqT_sl(KT - 1)[:, Bq : Bq + jt_n]
                ones = ocT_sl(0)[:, Bc : Bc + 1]

                # ---- Linear: u[o, j] = W @ o_q.T + b ----
                u = wpool.tile([128, OT, Bq], f16, tag=f"u{s}")
                for o in range(OT):
                    ups = ps_u.tile([128, Bq], f32, tag="ups")
                    for k in range(KT):
                        nc.tensor.matmul(
                            ups,
                            wt_k[k][:, o * 128 : (o + 1) * 128],
                            oqT_sl(k)[:, :Bq],
                            start=(k == 0),
                            stop=(k == KT - 1),
                        )
                    nc.vector.tensor_scalar(
                        out=u[:, o, :],
                        in0=ups,
                        scalar1=biasP[:, o : o + 1],
                        scalar2=None,
                        op0=mybir.AluOpType.add,
                    )

                # ---- score_T + exp: e[j, i] = exp((u.T @ o_cT)/32 + qb[j]),
                # denominator d[1, i] interleaved one tile behind ----
                dps = ps_d.tile([1, Bc], f32, tag="dmisc", name=f"dps_{s}")
                e_tiles = []
                e_rows = []
                for jt in range(jt_n):
                    mj = min(128, Bq - jt * 128)
                    sps = ps_s.tile([128, Bc], f32, tag="sps")
                    for o in range(OT):
                        nc.tensor.matmul(
                            sps[0:mj, :],
                            u[:, o, jt * 128 : jt * 128 + mj],
                            ocT_sl(o)[:, :Bc],
                            start=(o == 0),
                            stop=(o == OT - 1),
                        )
                    e = wpool.tile([128, Bc], f16, tag=f"e{s}_{jt}")
                    nc.scalar.activation(
                        out=e[0:mj, :],
                        in_=sps[0:mj, :],
                        func=mybir.ActivationFunctionType.Exp,
                        bias=qb[0:mj, jt : jt + 1],
                        scale=SCALE,
                    )
                    e_tiles.append(e)
                    e_rows.append(mj)
                    if jt >= 1:
                        pm = e_rows[jt - 1]
                        nc.tensor.matmul(
                            dps,
                            ones[0:pm, :],
                            e_tiles[jt - 1][0:pm, :],
                            start=(jt == 1),
                            stop=False,
                            skip_group_check=True,
                        )
                nc.tensor.matmul(
                    dps,
                    ones[0 : e_rows[-1], :],
                    e_tiles[-1][0 : e_rows[-1], :],
                    start=(jt_n == 1),
                    stop=True,
                    skip_group_check=True,
                )

                osb_tiles = {}

                def ctx_group(it, hb, mi):
                    if it not in osb_tiles:
                        osb_tiles[it] = opool.tile(
                            [128, H], f32, tag="osb", name=f"osb{it}_{s}"
                        )
                    cps = ps_c.tile([128, 512], f32, tag="cps", name=f"cps{it}{hb}_{s}")
                    for jt in range(jt_n):
                        kj = e_rows[jt]
                        nc.tensor.matmul(
                            cps[0:mi, :],
                            e_tiles[jt][0:kj, it * 128 : it * 128 + mi],
                            oqN[0:kj, jt, hb * 512 : (hb + 1) * 512],
                            start=(jt == 0),
                            stop=(jt == jt_n - 1),
                        )
                    return cps

                def ctx_evict(it, hb, mi, cps, r):
                    osb = osb_tiles[it]
                    nc.vector.tensor_scalar(
                        out=osb[0:mi, hb * 512 : (hb + 1) * 512],
                        in0=cps[0:mi, :],
                        scalar1=r[0:mi, :],
                        scalar2=None,
                        op0=mybir.AluOpType.mult,
                    )
                    nc.sync.dma_start(
                        out=out_d[
                            s, it * 128 : it * 128 + mi, hb * 512 : (hb + 1) * 512
                        ],
                        in_=osb[0:mi, hb * 512 : (hb + 1) * 512],
                    )

                mi0 = min(128, Bc)
                # first ctx group runs while the d copy drains on DVE
                cps00 = ctx_group(0, 0, mi0)
                dsb = wpool.tile([1, Bc], f32, tag=f"dsb{s}")
                nc.vector.tensor_copy(out=dsb, in_=dps)

                # transpose 1/d to per-partition columns via K=1 matmuls
                r_cols = []
                for it in range(it_n):
                    mi = min(128, Bc - it * 128)
                    dcps = ps_d.tile([128, 1], f32, tag="dmisc", name=f"dcps{it}_{s}")
                    nc.tensor.matmul(
                        dcps[0:mi, :],
                        dsb[:, it * 128 : it * 128 + mi],
                        ones_s[0:1, 0:1],
                        start=True,
                        stop=True,
                    )
                    r = wpool.tile([128, 1], f32, tag=f"r{s}_{it}")
                    nc.vector.reciprocal(out=r[0:mi, :], in_=dcps[0:mi, :])
                    r_cols.append(r)

                cps01 = ctx_group(0, 1, mi0)
                ctx_evict(0, 0, mi0, cps00, r_cols[0])
                ctx_evict(0, 1, mi0, cps01, r_cols[0])
                for it in range(1, it_n):
                    mi = min(128, Bc - it * 128)
                    for hb in range(HB):
                        cps = ctx_group(it, hb, mi)
                        ctx_evict(it, hb, mi, cps, r_cols[it])

    nc.compile()
    return nc


def _host_inputs(o_c, o_q, W, b, q_lengths, slots, budgets):
    """Build the per-core input maps (host-side sharding + re-layout)."""
    NEG16 = np.float16(-60000.0)  # exp(x - 60000) == 0 exactly in fp32
    wt_host = np.zeros((KT, 128, WTW), np.float16)
    wt_host[:, :, :H] = W.T.reshape(KT, 128, H)
    bias_host = np.ascontiguousarray(b.reshape(OT, 128).T)  # [128, o_tile] f32
    o_q16 = o_q.astype(np.float16)
    o_c16 = o_c.astype(np.float16)
    in_maps = []
    for c in range(N_CORES):
        oqT = np.zeros((N_SLOTS, 128, KT, QTW), np.float16)
        ocT = np.zeros((N_SLOTS, 128, KT, CTW), np.float16)
        oqN = np.zeros((N_SLOTS, 128, JT, H), np.float16)
        for s in range(N_SLOTS):
            g = slots[s][c]
            Bq, Bc = budgets[s]
            jt_n = max(1, _ceil_div(Bq, 128))
            # oqT[p, k, j] = o_q[j, k*128+p]
            oqT[s, :, :, :Tq] = o_q16[g].T.reshape(KT, 128, Tq).transpose(1, 0, 2)
            ocT[s, :, :, :Tc] = o_c16[g].T.reshape(KT, 128, Tc).transpose(1, 0, 2)
            # oqN[p, j, h] = o_q[j*128+p, h]
            oqN[s] = o_q16[g].reshape(JT, 128, H).transpose(1, 0, 2)
            ocT[s, :, 0, Bc] = 1.0  # ones column for the denominator matmul
            ql = int(q_lengths[g])
            jidx = np.arange(jt_n)[None, :] * 128 + np.arange(128)[:, None]
            oqT[s, :, KT - 1, Bq : Bq + jt_n] = np.where(
                jidx < ql, np.float16(0.0), NEG16
            )
        in_maps.append(
            {"oqT": oqT, "ocT": ocT, "oqN": oqN, "wt": wt_host, "biasP": bias_host}
        )
    return in_maps


def kernel(**inputs) -> np.ndarray:
    o_c = np.asarray(inputs["o_c"], dtype=np.float32)
    o_q = np.asarray(inputs["o_q"], dtype=np.float32)
    W = np.asarray(inputs["W"], dtype=np.float32)
    b = np.asarray(inputs["b"], dtype=np.float32)
    q_lengths = np.asarray(inputs["q_lengths"]).astype(np.int64)
    c_lengths = np.asarray(inputs["c_lengths"]).astype(np.int64)

    from concourse.bass_utils import run_bass_kernel_spmd

    slots, budgets = _plan(q_lengths, c_lengths)
    in_maps = _host_inputs(o_c, o_q, W, b, q_lengths, slots, budgets)
    nc = _build_program(budgets)

    trace = bool(int(os.environ.get("KERNEL_TRACE", "0")))
    res = run_bass_kernel_spmd(
        nc, in_maps, core_ids=list(range(N_CORES)), trace=trace
    )
    if trace:
        kernel.last_results = res

    out = np.zeros((B, Tc, H), dtype=np.float32)
    for c in range(N_CORES):
        dev = res.results[c]["out"]
        for s in range(N_SLOTS):
            g = slots[s][c]
            cl = int(c_lengths[g])
            out[g, :cl] = dev[s, :cl]
    return out


# revision 6
# speedup vs baseline: 1.3288x; 1.0908x over previous
"""C2Q attention kernel for 8 TRN2 NeuronCores, ragged-aware.

Math (per batch):
    u      = o_q @ W.T + b                       [Tq, H]
    score  = (o_c @ u.T) / sqrt(H)               [Tc, Tq]
    prob   = softmax_j(score masked at j>=q_len) [Tc, Tq]
    out    = (prob * (i < c_len)) @ o_q          [Tc, H]

Ragged scheme: lengths are in [Tq/2, Tq] x [Tc/2, Tc]. All 8 cores run one
SPMD program with 4 batch slots; slot s is compiled with budgets
Bq_s = max(q_len) / Bc_s = max(c_len) over the 8 batches assigned to that
slot (one per core). Batches are assigned to slots by a cost sort + local
search so budgets hug the actual lengths. Every matmul free dim and tile
count is sized to the slot budget:
    Linear  : 64 matmuls, N = Bq
    score   : 8 * jt_n matmuls, N = Bc, stationary M partial on last j tile
    context : 2 * it_n * jt_n matmuls, K partial on last j tile
    denom   : jt_n matmuls N = Bc; it_n K=1 transposes for 1/d
Device layout (everything K-on-partitions, no on-chip transposes):
    u computed as [o, j]  (lhsT = W.T[h, o] tile, rhs = o_qT[h, j])
    score computed TRANSPOSED e=[j, i] (lhsT = u[o, j-block], rhs = o_cT[o, i])
    exp via ACT with per-partition bias qb[j] in {0, -60000}: masked -> 0
    denominator d[1, i] = ones[j,1].T @ e  (matmul partition-reduce)
    1/d transposed to columns via K=1 matmuls, folded into context eviction
    context [i, h] = e[j, i-block].T @ o_q[j, h]
c_len row masking is applied host-side (rows >= c_len are never copied out).
"""

import os
import sys

import numpy as np

if "/opt/trn_rl_repo" not in sys.path:
    sys.path.insert(0, "/opt/trn_rl_repo")

B, Tc, Tq, H = 32, 512, 512, 1024
N_CORES = 8
N_SLOTS = B // N_CORES  # 4
KT = H // 128  # contraction tiles over h (8)
OT = H // 128  # linear-output tiles over o (8)
JT = Tq // 128  # max question-token tiles (4)
HB = H // 512  # free-dim blocks for context matmul (2)
SCALE = 1.0 / 32.0  # 1/sqrt(H)
WTW = H + 16  # wt slab width (pad)
QTW = Tq + 8  # oqT slab width (pad); qb rides at cols [Bq, Bq+jt_n)
CTW = Tc + 8  # ocT slab width (pad); ones column at col Bc of k=0 slab


def _ceil_div(a, b):
    return -(-a // b)


def _slot_cost(bq, bc):
    """Per-batch PE-time model (in 2.4GHz cycles) for budget (bq, bc)."""
    jt = max(1, _ceil_div(bq, 128))
    it = max(1, _ceil_div(bc, 128))
    lin = 64 * (bq + 112)
    score = 8 * jt * (bc + 112)
    ctx = 2 * it * jt * (512 + 112)
    den = jt * (bc + 16) + it * 382
    return lin + score + ctx + den


def _plan(q_lengths, c_lengths):
    """Assign batches to (core, slot) minimizing total slot-budget cost.

    Returns perm[slot][core] -> global batch idx, budgets[slot] = (Bq, Bc).
    """
    ql = np.clip(np.asarray(q_lengths, dtype=np.int64), 1, Tq)
    cl = np.clip(np.asarray(c_lengths, dtype=np.int64), 1, Tc)
    cost = np.array([_slot_cost(q, c) for q, c in zip(ql, cl)])
    order = np.argsort(cost, kind="stable")
    slots = [list(order[s * N_CORES : (s + 1) * N_CORES]) for s in range(N_SLOTS)]

    def total(sl):
        t = 0
        for members in sl:
            bq = max(int(ql[b]) for b in members)
            bc = max(int(cl[b]) for b in members)
            t += _slot_cost(bq, bc)
        return t

    best = total(slots)
    improved = True
    while improved:
        improved = False
        for s1 in range(N_SLOTS):
            for s2 in range(s1 + 1, N_SLOTS):
                for i in range(N_CORES):
                    for j in range(N_CORES):
                        a, b_ = slots[s1][i], slots[s2][j]
                        slots[s1][i], slots[s2][j] = b_, a
                        t = total(slots)
                        if t < best:
                            best = t
                            improved = True
                        else:
                            slots[s1][i], slots[s2][j] = a, b_
    budgets = []
    for members in slots:
        bq = max(int(ql[b]) for b in members)
        bc = max(int(cl[b]) for b in members)
        budgets.append((bq, bc))
    # order slots so the cheapest runs first (fast ramp) -- keep budgets
    # aligned with membership
    idx = sorted(range(N_SLOTS), key=lambda s: _slot_cost(*budgets[s]))
    slots = [slots[s] for s in idx]
    budgets = [budgets[s] for s in idx]
    return slots, budgets


def _build_program(budgets):
    import concourse.bacc as bacc
    import concourse.mybir as mybir
    import concourse.tile as tile

    f32 = mybir.dt.float32
    f16 = mybir.dt.float16

    nc = bacc.Bacc("TRN2", debug=False)

    oqT_d = nc.declare_dram_parameter("oqT", [N_SLOTS, 128, KT, QTW], f16, isOutput=False)
    ocT_d = nc.declare_dram_parameter("ocT", [N_SLOTS, 128, KT, CTW], f16, isOutput=False)
    oqN_d = nc.declare_dram_parameter("oqN", [N_SLOTS, 128, JT, H], f16, isOutput=False)
    wt_d = nc.declare_dram_parameter("wt", [KT, 128, WTW], f16, isOutput=False)
    bias_d = nc.declare_dram_parameter("biasP", [128, OT], f32, isOutput=False)
    out_d = nc.declare_dram_parameter("out", [N_SLOTS, Tc, H], f16, isOutput=True)

    with tile.TileContext(nc) as tc:
        with (
            tc.tile_pool(name="const", bufs=1) as cpool,
            tc.tile_pool(name="inp", bufs=1) as ipool,
            tc.tile_pool(name="work", bufs=1) as wpool,
            tc.tile_pool(name="outp", bufs=3) as opool,
            tc.tile_pool(name="ps_u", bufs=2, space="PSUM") as ps_u,
            tc.tile_pool(name="ps_s", bufs=2, space="PSUM") as ps_s,
            tc.tile_pool(name="ps_c", bufs=3, space="PSUM") as ps_c,
            tc.tile_pool(name="ps_d", bufs=1, space="PSUM") as ps_d,
        ):
            ones_s = cpool.tile([1, 1], f32)
            nc.vector.memset(ones_s, 1.0)

            wt_k = [
                cpool.tile([128, WTW], f16, tag=f"wt{k}", name=f"wt{k}")
                for k in range(KT)
            ]
            biasP = cpool.tile([128, OT], f32)

            # --- per-slot geometry + input tiles, all DMAs issued upfront ---
            geo = []
            for s, (Bq, Bc) in enumerate(budgets):
                jt_n = max(1, _ceil_div(Bq, 128))
                it_n = max(1, _ceil_div(Bc, 128))
                QW = Bq + 8
                CW = Bc + 8
                if s == 0:
                    # per-k tiles keep DMA->matmul deps fine-grained during ramp
                    oqT = [
                        ipool.tile([128, QW], f16, tag=f"oqT0_{k}", name=f"oqT0_{k}")
                        for k in range(KT)
                    ]
                    ocT = [
                        ipool.tile([128, CW], f16, tag=f"ocT0_{k}", name=f"ocT0_{k}")
                        for k in range(KT)
                    ]
                else:
                    oqT = ipool.tile([128, KT, QW], f16, tag=f"oqT{s}")
                    ocT = ipool.tile([128, KT, CW], f16, tag=f"ocT{s}")
                oqN = ipool.tile([128, jt_n, H], f16, tag=f"oqN{s}")
                geo.append((Bq, Bc, jt_n, it_n, oqT, ocT, oqN))

            # DMA order: interleaved wt/oqT(slot0) with bias third (bias is
            # only needed at the first Linear eviction), ocT(slot0), oqN(0),
            # then merged slabs for the remaining slots.
            Bq0, Bc0, jt0, _, oqT0, ocT0, oqN0 = geo[0]
            for k in range(KT):
                nc.sync.dma_start(out=wt_k[k], in_=wt_d[k])
                nc.sync.dma_start(out=oqT0[k], in_=oqT_d[0, :, k, : Bq0 + 8])
                if k == 0:
                    nc.sync.dma_start(out=biasP, in_=bias_d[:, :])
            for k in range(KT):
                nc.sync.dma_start(out=ocT0[k], in_=ocT_d[0, :, k, : Bc0 + 8])
            nc.sync.dma_start(out=oqN0, in_=oqN_d[0, :, :jt0, :])
            for s in range(1, N_SLOTS):
                Bq, Bc, jt_n, it_n, oqT, ocT, oqN = geo[s]
                nc.sync.dma_start(out=oqT, in_=oqT_d[s, :, :, : Bq + 8])
                nc.sync.dma_start(out=ocT, in_=ocT_d[s, :, :, : Bc + 8])
                nc.sync.dma_start(out=oqN, in_=oqN_d[s, :, :jt_n, :])

            for s in range(N_SLOTS):
                Bq, Bc, jt_n, it_n, oqT, ocT, oqN = geo[s]

                def oqT_sl(k):
                    return oqT[k] if s == 0 else oqT[:, k, :]

                def ocT_sl(k):
                    return ocT[k] if s == 0 else ocT[:, k, :]

                qb = oqT_sl(KT - 1)[:, Bq : Bq + jt_n]
                ones = ocT_sl(0)[:, Bc : Bc + 1]

                # ---- Linear: u[o, j] = W @ o_q.T + b ----
                u = wpool.tile([128, OT, Bq], f16, tag=f"u{s}")
                for o in range(OT):
                    ups = ps_u.tile([128, Bq], f32, tag="ups")
                    for k in range(KT):
                        nc.tensor.matmul(
                            ups,
                            wt_k[k][:, o * 128 : (o + 1) * 128],
                            oqT_sl(k)[:, :Bq],
                            start=(k == 0),
                            stop=(k == KT - 1),
                        )
                    nc.vector.tensor_scalar(
                        out=u[:, o, :],
                        in0=ups,
                        scalar1=biasP[:, o : o + 1],
                        scalar2=None,
                        op0=mybir.AluOpType.add,
                    )

                # ---- score_T + exp: e[j, i] = exp((u.T @ o_cT)/32 + qb[j]),
                # denominator d[1, i] interleaved one tile behind ----
                dps = ps_d.tile([1, Bc], f32, tag="dmisc", name=f"dps_{s}")
                e_tiles = []
                e_rows = []
                for jt in range(jt_n):
                    mj = min(128, Bq - jt * 128)
                    sps = ps_s.tile([128, Bc], f32, tag="sps")
                    for o in range(OT):
                        nc.tensor.matmul(
                            sps[0:mj, :],
                            u[:, o, jt * 128 : jt * 128 + mj],
                            ocT_sl(o)[:, :Bc],
                            start=(o == 0),
                            stop=(o == OT - 1),
                        )
                    e = wpool.tile([128, Bc], f16, tag=f"e{s}_{jt}")
                    nc.scalar.activation(
                        out=e[0:mj, :],
                        in_=sps[0:mj, :],
                        func=mybir.ActivationFunctionType.Exp,
                        bias=qb[0:mj, jt : jt + 1],
                        scale=SCALE,
                    )
                    e_tiles.append(e)
                    e_rows.append(mj)
                    if jt >= 1:
                        pm = e_rows[jt - 1]
                        nc.tensor.matmul(
                            dps,
                            ones[0:pm, :],
                            e_tiles[jt - 1][0:pm, :],
                            start=(jt == 1),
                            stop=False,
                            skip_group_check=True,
                        )
                nc.tensor.matmul(
                    dps,
                    ones[0 : e_rows[-1], :],
                    e_tiles[-1][0 : e_rows[-1], :],
                    start=(jt_n == 1),
                    stop=True,
                    skip_group_check=True,
                )

                osb_tiles = {}

                def ctx_group_pair(it, mi):
                    """Both hb halves for one i-tile; the second matmul of
                    each jt step reuses the stationary e-block loaded by the
                    first (ldweights=False)."""
                    if it not in osb_tiles:
                        osb_tiles[it] = opool.tile(
                            [128, H], f16, tag="osb", name=f"osb{it}_{s}"
                        )
                    cps = [
                        ps_c.tile([128, 512], f32, tag="cps", name=f"cps{it}{hb}_{s}")
                        for hb in range(HB)
                    ]
                    for jt in range(jt_n):
                        kj = e_rows[jt]
                        for hb in range(HB):
                            inst = nc.tensor.matmul(
                                cps[hb][0:mi, :],
                                e_tiles[jt][0:kj, it * 128 : it * 128 + mi],
                                oqN[0:kj, jt, hb * 512 : (hb + 1) * 512],
                                start=(jt == 0),
                                stop=(jt == jt_n - 1),
                            )
                            if hb > 0:
                                inst.ins.ldweights = False
                    return cps

                def ctx_evict(it, hb, mi, cps, r):
                    osb = osb_tiles[it]
                    nc.vector.tensor_scalar(
                        out=osb[0:mi, hb * 512 : (hb + 1) * 512],
                        in0=cps[0:mi, :],
                        scalar1=r[0:mi, :],
                        scalar2=None,
                        op0=mybir.AluOpType.mult,
                    )
                    nc.sync.dma_start(
                        out=out_d[
                            s, it * 128 : it * 128 + mi, hb * 512 : (hb + 1) * 512
                        ],
                        in_=osb[0:mi, hb * 512 : (hb + 1) * 512],
                    )

                mi0 = min(128, Bc)
                # first ctx group runs while the d copy drains on DVE
                cps0 = ctx_group_pair(0, mi0)
                dsb = wpool.tile([1, Bc], f32, tag=f"dsb{s}")
                nc.vector.tensor_copy(out=dsb, in_=dps)

                # transpose 1/d to per-partition columns via K=1 matmuls
                r_cols = []
                for it in range(it_n):
                    mi = min(128, Bc - it * 128)
                    dcps = ps_d.tile([128, 1], f32, tag="dmisc", name=f"dcps{it}_{s}")
                    nc.tensor.matmul(
                        dcps[0:mi, :],
                        dsb[:, it * 128 : it * 128 + mi],
                        ones_s[0:1, 0:1],
                        start=True,
                        stop=True,
                    )
                    r = wpool.tile([128, 1], f32, tag=f"r{s}_{it}")
                    nc.vector.reciprocal(out=r[0:mi, :], in_=dcps[0:mi, :])
                    r_cols.append(r)

                ctx_evict(0, 0, mi0, cps0[0], r_cols[0])
                ctx_evict(0, 1, mi0, cps0[1], r_cols[0])
                for it in range(1, it_n):
                    mi = min(128, Bc - it * 128)
                    cps = ctx_group_pair(it, mi)
                    for hb in range(HB):
                        ctx_evict(it, hb, mi, cps[hb], r_cols[it])

    nc.compile()
    return nc


def _host_inputs(o_c, o_q, W, b, q_lengths, slots, budgets):
    """Build the per-core input maps (host-side sharding + re-layout)."""
    NEG16 = np.float16(-60000.0)  # exp(x - 60000) == 0 exactly in fp32
    wt_host = np.zeros((KT, 128, WTW), np.float16)
    wt_host[:, :, :H] = W.T.reshape(KT, 128, H)
    bias_host = np.ascontiguousarray(b.reshape(OT, 128).T)  # [128, o_tile] f32
    o_q16 = o_q.astype(np.float16)
    o_c16 = o_c.astype(np.float16)
    in_maps = []
    for c in range(N_CORES):
        oqT = np.zeros((N_SLOTS, 128, KT, QTW), np.float16)
        ocT = np.zeros((N_SLOTS, 128, KT, CTW), np.float16)
        oqN = np.zeros((N_SLOTS, 128, JT, H), np.float16)
        for s in range(N_SLOTS):
            g = slots[s][c]
            Bq, Bc = budgets[s]
            jt_n = max(1, _ceil_div(Bq, 128))
            # oqT[p, k, j] = o_q[j, k*128+p]
            oqT[s, :, :, :Tq] = o_q16[g].T.reshape(KT, 128, Tq).transpose(1, 0, 2)
            ocT[s, :, :, :Tc] = o_c16[g].T.reshape(KT, 128, Tc).transpose(1, 0, 2)
            # oqN[p, j, h] = o_q[j*128+p, h]
            oqN[s] = o_q16[g].reshape(JT, 128, H).transpose(1, 0, 2)
            ocT[s, :, 0, Bc] = 1.0  # ones column for the denominator matmul
            ql = int(q_lengths[g])
            jidx = np.arange(jt_n)[None, :] * 128 + np.arange(128)[:, None]
            oqT[s, :, KT - 1, Bq : Bq + jt_n] = np.where(
                jidx < ql, np.float16(0.0), NEG16
            )
        in_maps.append(
            {"oqT": oqT, "ocT": ocT, "oqN": oqN, "wt": wt_host, "biasP": bias_host}
        )
    return in_maps


def kernel(**inputs) -> np.ndarray:
    o_c = np.asarray(inputs["o_c"], dtype=np.float32)
    o_q = np.asarray(inputs["o_q"], dtype=np.float32)
    W = np.asarray(inputs["W"], dtype=np.float32)
    b = np.asarray(inputs["b"], dtype=np.float32)
    q_lengths = np.asarray(inputs["q_lengths"]).astype(np.int64)
    c_lengths = np.asarray(inputs["c_lengths"]).astype(np.int64)

    from concourse.bass_utils import run_bass_kernel_spmd

    slots, budgets = _plan(q_lengths, c_lengths)
    in_maps = _host_inputs(o_c, o_q, W, b, q_lengths, slots, budgets)
    nc = _build_program(budgets)

    trace = bool(int(os.environ.get("KERNEL_TRACE", "0")))
    res = run_bass_kernel_spmd(
        nc, in_maps, core_ids=list(range(N_CORES)), trace=trace
    )
    if trace:
        kernel.last_results = res

    out = np.zeros((B, Tc, H), dtype=np.float32)
    for c in range(N_CORES):
        dev = res.results[c]["out"]
        for s in range(N_SLOTS):
            g = slots[s][c]
            cl = int(c_lengths[g])
            out[g, :cl] = dev[s, :cl]
    return out


# revision 15
# speedup vs baseline: 1.3883x; 1.0448x over previous
"""C2Q attention kernel for 8 TRN2 NeuronCores, ragged-aware.

Math (per batch):
    u      = o_q @ W.T + b                       [Tq, H]
    score  = (o_c @ u.T) / sqrt(H)               [Tc, Tq]
    prob   = softmax_j(score masked at j>=q_len) [Tc, Tq]
    out    = (prob * (i < c_len)) @ o_q          [Tc, H]

Ragged scheme: lengths are in [Tq/2, Tq] x [Tc/2, Tc]. All 8 cores run one
SPMD program with 4 batch slots; slot s is compiled with budgets
Bq_s = max(q_len) / Bc_s = max(c_len) over the 8 batches assigned to that
slot (one per core). Batches are assigned to slots by a cost sort + local
search so budgets hug the actual lengths. Every matmul free dim and tile
count is sized to the slot budget:
    Linear  : 64 matmuls, N = Bq
    score   : 8 * jt_n matmuls, N = Bc, stationary M partial on last j tile
    context : 2 * it_n * jt_n matmuls, K partial on last j tile
    denom   : jt_n matmuls N = Bc; it_n K=1 transposes for 1/d
Device layout (everything K-on-partitions, no on-chip transposes):
    u computed as [o, j]  (lhsT = W.T[h, o] tile, rhs = o_qT[h, j])
    score computed TRANSPOSED e=[j, i] (lhsT = u[o, j-block], rhs = o_cT[o, i])
    exp via ACT with per-partition bias qb[j] in {0, -60000}: masked -> 0
    denominator d[1, i] = ones[j,1].T @ e  (matmul partition-reduce)
    1/d transposed to columns via K=1 matmuls, folded into context eviction
    context [i, h] = e[j, i-block].T @ o_q[j, h]
c_len row masking is applied host-side (rows >= c_len are never copied out).
"""

import os
import sys

import numpy as np

if "/opt/trn_rl_repo" not in sys.path:
    sys.path.insert(0, "/opt/trn_rl_repo")

B, Tc, Tq, H = 32, 512, 512, 1024
N_CORES = 8
N_SLOTS = B // N_CORES  # 4
KT = H // 128  # contraction tiles over h (8)
OT = H // 128  # linear-output tiles over o (8)
JT = Tq // 128  # max question-token tiles (4)
HB = H // 512  # free-dim blocks for context matmul (2)
SCALE = 1.0 / 32.0  # 1/sqrt(H)
WTW = H + 16  # wt slab width (pad)
QTW = Tq + 8  # oqT slab width (pad); qb rides at cols [Bq, Bq+jt_n)
CTW = Tc + 8  # ocT slab width (pad); ones column at col Bc of k=0 slab


def _ceil_div(a, b):
    return -(-a // b)


def _slot_cost(bq, bc):
    """Per-batch PE-time model (in 2.4GHz cycles) for budget (bq, bc)."""
    jt = max(1, _ceil_div(bq, 128))
    it = max(1, _ceil_div(bc, 128))
    lin = 64 * (bq + 112)
    score = 8 * jt * (bc + 112)
    ctx = 2 * it * jt * (512 + 112)
    den = jt * (bc + 16) + it * 382
    return lin + score + ctx + den


def _plan(q_lengths, c_lengths):
    """Assign batches to (core, slot) minimizing total slot-budget cost.

    Returns perm[slot][core] -> global batch idx, budgets[slot] = (Bq, Bc).
    """
    ql = np.clip(np.asarray(q_lengths, dtype=np.int64), 1, Tq)
    cl = np.clip(np.asarray(c_lengths, dtype=np.int64), 1, Tc)
    cost = np.array([_slot_cost(q, c) for q, c in zip(ql, cl)])
    order = np.argsort(cost, kind="stable")
    slots = [list(order[s * N_CORES : (s + 1) * N_CORES]) for s in range(N_SLOTS)]

    def total(sl):
        t = 0
        for members in sl:
            bq = max(int(ql[b]) for b in members)
            bc = max(int(cl[b]) for b in members)
            t += _slot_cost(bq, bc)
        return t

    best = total(slots)
    improved = True
    while improved:
        improved = False
        for s1 in range(N_SLOTS):
            for s2 in range(s1 + 1, N_SLOTS):
                for i in range(N_CORES):
                    for j in range(N_CORES):
                        a, b_ = slots[s1][i], slots[s2][j]
                        slots[s1][i], slots[s2][j] = b_, a
                        t = total(slots)
                        if t < best:
                            best = t
                            improved = True
                        else:
                            slots[s1][i], slots[s2][j] = a, b_
    budgets = []
    for members in slots:
        bq = max(int(ql[b]) for b in members)
        bc = max(int(cl[b]) for b in members)
        budgets.append((bq, bc))
    # order slots so the cheapest runs first (fast ramp) -- keep budgets
    # aligned with membership
    idx = sorted(range(N_SLOTS), key=lambda s: _slot_cost(*budgets[s]))
    slots = [slots[s] for s in idx]
    budgets = [budgets[s] for s in idx]
    return slots, budgets


def _build_program(budgets):
    import concourse.bacc as bacc
    import concourse.mybir as mybir
    import concourse.tile as tile

    f32 = mybir.dt.float32
    f16 = mybir.dt.float16

    nc = bacc.Bacc("TRN2", debug=False)

    oqT_d = nc.declare_dram_parameter("oqT", [N_SLOTS, 128, KT, QTW], f16, isOutput=False)
    ocT_d = nc.declare_dram_parameter("ocT", [N_SLOTS, 128, KT, CTW], f16, isOutput=False)
    oqN_d = nc.declare_dram_parameter("oqN", [N_SLOTS, 128, JT, H], f16, isOutput=False)
    wt_d = nc.declare_dram_parameter("wt", [128, KT, WTW], f16, isOutput=False)
    bias_d = nc.declare_dram_parameter("biasP", [128, OT], f32, isOutput=False)
    out_d = nc.declare_dram_parameter("out", [N_SLOTS, Tc, H], f16, isOutput=True)

    with tile.TileContext(nc) as tc:
        with (
            tc.tile_pool(name="const", bufs=1) as cpool,
            tc.tile_pool(name="inp", bufs=1) as ipool,
            tc.tile_pool(name="work", bufs=1) as wpool,
            tc.tile_pool(name="outp", bufs=3) as opool,
            tc.tile_pool(name="ps_u", bufs=2, space="PSUM") as ps_u,
            tc.tile_pool(name="ps_s", bufs=2, space="PSUM") as ps_s,
            tc.tile_pool(name="ps_c", bufs=3, space="PSUM") as ps_c,
            tc.tile_pool(name="ps_d", bufs=1, space="PSUM") as ps_d,
        ):
            ones_s = cpool.tile([1, 1], f32)
            nc.vector.memset(ones_s, 1.0)

            wt = cpool.tile([128, KT, WTW], f16, tag="wt", name="wt")
            biasP = cpool.tile([128, OT], f32)

            # --- per-slot geometry + input tiles, all DMAs issued upfront ---
            geo = []
            for s, (Bq, Bc) in enumerate(budgets):
                jt_n = max(1, _ceil_div(Bq, 128))
                it_n = max(1, _ceil_div(Bc, 128))
                QW = Bq + 8
                CW = Bc + 8
                oqT = ipool.tile([128, KT, QW], f16, tag=f"oqT{s}")
                ocT = ipool.tile([128, KT, CW], f16, tag=f"ocT{s}")
                oqN = ipool.tile([128, jt_n, H], f16, tag=f"oqN{s}")
                geo.append((Bq, Bc, jt_n, it_n, oqT, ocT, oqN))

            # DMA order: slot-0 wt/oqT split in k-ranges (0, 1:4, 4:8) so the
            # ramp-critical Linear can start after ~360KB; bias rides third
            # (only needed at the first Linear eviction). Everything else
            # merged, one trigger per slab (the Sync trigger stream is serial
            # at ~0.6us per dma_start).
            Bq0, Bc0, jt0, _, oqT0, ocT0, oqN0 = geo[0]
            for lo, hi in ((0, 1), (1, 4), (4, 8)):
                nc.sync.dma_start(out=wt[:, lo:hi, :], in_=wt_d[:, lo:hi, :])
                nc.sync.dma_start(
                    out=oqT0[:, lo:hi, :], in_=oqT_d[0, :, lo:hi, : Bq0 + 8]
                )
                if lo == 0:
                    nc.sync.dma_start(out=biasP, in_=bias_d[:, :])
            nc.sync.dma_start(out=ocT0, in_=ocT_d[0, :, :, : Bc0 + 8])
            nc.sync.dma_start(out=oqN0, in_=oqN_d[0, :, :jt0, :])
            for s in range(1, N_SLOTS):
                Bq, Bc, jt_n, it_n, oqT, ocT, oqN = geo[s]
                nc.sync.dma_start(out=oqT, in_=oqT_d[s, :, :, : Bq + 8])
                nc.sync.dma_start(out=ocT, in_=ocT_d[s, :, :, : Bc + 8])
                nc.sync.dma_start(out=oqN, in_=oqN_d[s, :, :jt_n, :])

            for s in range(N_SLOTS):
                Bq, Bc, jt_n, it_n, oqT, ocT, oqN = geo[s]

                qb = oqT[:, KT - 1, Bq : Bq + jt_n]
                ones = ocT[:, 0, Bc : Bc + 1]

                # ---- Linear: u[o, j] = W @ o_q.T + b ----
                u = wpool.tile([128, OT, Bq], f16, tag=f"u{s}")
                if s == 0:
                    # k-outer with 8 open PSUM o-groups (banks borrowed from
                    # every pool -- nothing else is in PSUM yet): each wt/oqT
                    # k-slab is consumed the moment its DMA lands, so the
                    # ramp is gated by the DMA trigger stream, not by
                    # o-group serialization.
                    pools8 = [ps_u, ps_u, ps_s, ps_s, ps_c, ps_c, ps_c, ps_d]
                    tags8 = ["ups", "ups", "sps", "sps", "cps", "cps", "cps", "dmisc"]
                    upss = [
                        pools8[o].tile(
                            [128, Bq], f32, tag=tags8[o], name=f"ups0_{o}"
                        )
                        for o in range(OT)
                    ]
                    for k in range(KT):
                        for o in range(OT):
                            nc.tensor.matmul(
                                upss[o],
                                wt[:, k, o * 128 : (o + 1) * 128],
                                oqT[:, k, :Bq],
                                start=(k == 0),
                                stop=(k == KT - 1),
                            )
                    for o in range(OT):
                        nc.vector.tensor_scalar(
                            out=u[:, o, :],
                            in0=upss[o],
                            scalar1=biasP[:, o : o + 1],
                            scalar2=None,
                            op0=mybir.AluOpType.add,
                        )
                else:
                    for o in range(OT):
                        ups = ps_u.tile([128, Bq], f32, tag="ups")
                        for k in range(KT):
                            nc.tensor.matmul(
                                ups,
                                wt[:, k, o * 128 : (o + 1) * 128],
                                oqT[:, k, :Bq],
                                start=(k == 0),
                                stop=(k == KT - 1),
                            )
                        nc.vector.tensor_scalar(
                            out=u[:, o, :],
                            in0=ups,
                            scalar1=biasP[:, o : o + 1],
                            scalar2=None,
                            op0=mybir.AluOpType.add,
                        )

                # ---- score_T + exp: e[j, i] = exp((u.T @ o_cT)/32 + qb[j]).
                # The e tiles are pre-summed on DVE (esum) so the denominator
                # d[1, i] needs a single partition-reduce matmul instead of
                # jt_n of them. Rows [kj, 128) of esum hold the full-tile
                # partial sums only, which is exactly right: the last tile's
                # missing rows don't exist as tokens.
                e_tiles = []
                e_rows = []
                esum = wpool.tile([128, Bc], f16, tag=f"esum{s}")
                for jt in range(jt_n):
                    mj = min(128, Bq - jt * 128)
                    sps = ps_s.tile([128, Bc], f32, tag="sps")
                    for o in range(OT):
                        nc.tensor.matmul(
                            sps[0:mj, :],
                            u[:, o, jt * 128 : jt * 128 + mj],
                            ocT[:, o, :Bc],
                            start=(o == 0),
                            stop=(o == OT - 1),
                        )
                    e = wpool.tile([128, Bc], f16, tag=f"e{s}_{jt}")
                    nc.scalar.activation(
                        out=e[0:mj, :],
                        in_=sps[0:mj, :],
                        func=mybir.ActivationFunctionType.Exp,
                        bias=qb[0:mj, jt : jt + 1],
                        scale=SCALE,
                    )
                    e_tiles.append(e)
                    e_rows.append(mj)
                    if jt == 1:
                        nc.vector.tensor_tensor(
                            out=esum[0 : e_rows[1], :],
                            in0=e_tiles[0][0 : e_rows[1], :],
                            in1=e_tiles[1][0 : e_rows[1], :],
                            op=mybir.AluOpType.add,
                        )
                        if e_rows[1] < 128:
                            nc.vector.tensor_copy(
                                out=esum[e_rows[1] : 128, :],
                                in_=e_tiles[0][e_rows[1] : 128, :],
                            )
                    elif jt >= 2:
                        nc.vector.tensor_tensor(
                            out=esum[0:mj, :],
                            in0=esum[0:mj, :],
                            in1=e[0:mj, :],
                            op=mybir.AluOpType.add,
                        )
                osb_tiles = {}

                def ctx_group_pair(it, mi):
                    """Both hb halves for one i-tile; the second matmul of
                    each jt step reuses the stationary e-block loaded by the
                    first (ldweights=False)."""
                    if it not in osb_tiles:
                        osb_tiles[it] = opool.tile(
                            [128, H], f16, tag="osb", name=f"osb{it}_{s}"
                        )
                    cps = [
                        ps_c.tile([128, 512], f32, tag="cps", name=f"cps{it}{hb}_{s}")
                        for hb in range(HB)
                    ]
                    for jt in range(jt_n):
                        kj = e_rows[jt]
                        for hb in range(HB):
                            inst = nc.tensor.matmul(
                                cps[hb][0:mi, :],
                                e_tiles[jt][0:kj, it * 128 : it * 128 + mi],
                                oqN[0:kj, jt, hb * 512 : (hb + 1) * 512],
                                start=(jt == 0),
                                stop=(jt == jt_n - 1),
                            )
                            if hb > 0:
                                inst.ins.ldweights = False
                    return cps

                def ctx_evict(it, hb, mi, cps, r):
                    osb = osb_tiles[it]
                    nc.vector.tensor_scalar(
                        out=osb[0:mi, hb * 512 : (hb + 1) * 512],
                        in0=cps[0:mi, :],
                        scalar1=r[0:mi, :],
                        scalar2=None,
                        op0=mybir.AluOpType.mult,
                    )
                    if hb == HB - 1:
                        nc.sync.dma_start(
                            out=out_d[s, it * 128 : it * 128 + mi, :],
                            in_=osb[0:mi, :],
                        )

                mi0 = min(128, Bc)
                # first ctx group runs while DVE finishes esum; then the
                # single partition-reduce for the denominator
                cps0 = ctx_group_pair(0, mi0)
                dps = ps_d.tile([1, Bc], f32, tag="dmisc", name=f"dps_{s}")
                dsrc = esum if jt_n > 1 else e_tiles[0]
                drows = 128 if jt_n > 1 else e_rows[0]
                nc.tensor.matmul(
                    dps,
                    ones[0:drows, :],
                    dsrc[0:drows, :],
                    start=True,
                    stop=True,
                )
                dsb = wpool.tile([1, Bc], f32, tag=f"dsb{s}")
                nc.vector.tensor_copy(out=dsb, in_=dps)

                # transpose 1/d to per-partition columns via K=1 matmuls
                r_cols = []
                for it in range(it_n):
                    mi = min(128, Bc - it * 128)
                    dcps = ps_d.tile([128, 1], f32, tag="dmisc", name=f"dcps{it}_{s}")
                    nc.tensor.matmul(
                        dcps[0:mi, :],
                        dsb[:, it * 128 : it * 128 + mi],
                        ones_s[0:1, 0:1],
                        start=True,
                        stop=True,
                    )
                    r = wpool.tile([128, 1], f32, tag=f"r{s}_{it}")
                    nc.vector.reciprocal(out=r[0:mi, :], in_=dcps[0:mi, :])
                    r_cols.append(r)

                ctx_evict(0, 0, mi0, cps0[0], r_cols[0])
                ctx_evict(0, 1, mi0, cps0[1], r_cols[0])
                for it in range(1, it_n):
                    mi = min(128, Bc - it * 128)
                    cps = ctx_group_pair(it, mi)
                    for hb in range(HB):
                        ctx_evict(it, hb, mi, cps[hb], r_cols[it])

    nc.compile()
    return nc


def _host_inputs(o_c, o_q, W, b, q_lengths, slots, budgets):
    """Build the per-core input maps (host-side sharding + re-layout)."""
    NEG16 = np.float16(-60000.0)  # exp(x - 60000) == 0 exactly in fp32
    # wt[p, k, col] = W.T[k*128+p, col] (partition-major slab)
    wt_host = np.zeros((128, KT, WTW), np.float16)
    wt_host[:, :, :H] = W.T.reshape(KT, 128, H).transpose(1, 0, 2)
    bias_host = np.ascontiguousarray(b.reshape(OT, 128).T)  # [128, o_tile] f32
    o_q16 = o_q.astype(np.float16)
    o_c16 = o_c.astype(np.float16)
    in_maps = []
    for c in range(N_CORES):
        oqT = np.zeros((N_SLOTS, 128, KT, QTW), np.float16)
        ocT = np.zeros((N_SLOTS, 128, KT, CTW), np.float16)
        oqN = np.zeros((N_SLOTS, 128, JT, H), np.float16)
        for s in range(N_SLOTS):
            g = slots[s][c]
            Bq, Bc = budgets[s]
            jt_n = max(1, _ceil_div(Bq, 128))
            # oqT[p, k, j] = o_q[j, k*128+p]
            oqT[s, :, :, :Tq] = o_q16[g].T.reshape(KT, 128, Tq).transpose(1, 0, 2)
            ocT[s, :, :, :Tc] = o_c16[g].T.reshape(KT, 128, Tc).transpose(1, 0, 2)
            # oqN[p, j, h] = o_q[j*128+p, h]
            oqN[s] = o_q16[g].reshape(JT, 128, H).transpose(1, 0, 2)
            ocT[s, :, 0, Bc] = 1.0  # ones column for the denominator matmul
            ql = int(q_lengths[g])
            jidx = np.arange(jt_n)[None, :] * 128 + np.arange(128)[:, None]
            oqT[s, :, KT - 1, Bq : Bq + jt_n] = np.where(
                jidx < ql, np.float16(0.0), NEG16
            )
        in_maps.append(
            {"oqT": oqT, "ocT": ocT, "oqN": oqN, "wt": wt_host, "biasP": bias_host}
        )
    return in_maps


def kernel(**inputs) -> np.ndarray:
    o_c = np.asarray(inputs["o_c"], dtype=np.float32)
    o_q = np.asarray(inputs["o_q"], dtype=np.float32)
    W = np.asarray(inputs["W"], dtype=np.float32)
    b = np.asarray(inputs["b"], dtype=np.float32)
    q_lengths = np.asarray(inputs["q_lengths"]).astype(np.int64)
    c_lengths = np.asarray(inputs["c_lengths"]).astype(np.int64)

    from concourse.bass_utils import run_bass_kernel_spmd

    slots, budgets = _plan(q_lengths, c_lengths)
    in_maps = _host_inputs(o_c, o_q, W, b, q_lengths, slots, budgets)
    nc = _build_program(budgets)

    trace = bool(int(os.environ.get("KERNEL_TRACE", "0")))
    res = run_bass_kernel_spmd(
        nc, in_maps, core_ids=list(range(N_CORES)), trace=trace
    )
    if trace:
        kernel.last_results = res

    out = np.zeros((B, Tc, H), dtype=np.float32)
    for c in range(N_CORES):
        dev = res.results[c]["out"]
        for s in range(N_SLOTS):
            g = slots[s][c]
            cl = int(c_lengths[g])
            out[g, :cl] = dev[s, :cl]
    return out
